# revision 4
# baseline (speedup 1.0000x reference)
"""Trainium2 Bass kernel for DiscreteKeyValueBottleneck (vq_codebook gather).

out[b, n, h, :] = values[h, idx[b, n, h], :], flattened to (B, N, H*D).

Sharding: data-parallel over batch across 8 NeuronCores (8 batches/core).

Device strategy (per core):
  - values are padded host-side to a 256B-strided table values_pad[H*C, 64]
    (only [:, :12] used) living in DRAM.
  - The gather uses the GPSIMD SWDGE `InstDMAGatherAnt` (MoE gather ucode,
    library `mlp`), elem_size=12 f32 (48B) with elem_step=64 (256B stride).
    int16 indices limit one instruction to 32768 table rows = 8 heads, so
    heads are processed in 8 groups. Index order per instruction is
    j = (g*8 + hh)*128 + p  ->  output slot [p, g*8+hh], which lays each
    token's 8-head segment (96 f32, 384B) contiguously in SBUF.
  - Stores: per g-group, [128 tokens, 96] SBUF -> DRAM rows (strided cols).
Indices are marshaled on host into the wrapped int16 layout the ucode
expects ([16, S] wrapped, replicated across the 8 Q7 core groups).
"""

import numpy as np

import concourse.bass as bass
import concourse.mybir as mybir
import concourse.tile as tile
from concourse import bacc, library_config
from concourse._compat import exact_div
from concourse.bass_utils import run_bass_kernel_spmd

# Problem constants (hardcoded per harness contract)
B, N, H, C, D = 64, 2048, 64, 4096, 12
N_CORES = 8
B_PER_CORE = B // N_CORES          # 8
T = B_PER_CORE * N                 # 16384 tokens per core
F = H * D                          # 768 features per token
P = 128
PAD = 64                           # padded table row (f32) -> 256B stride

TOK = 1024                         # tokens per gather instruction
HG = 8                             # heads per group (8 groups of 8)
NUM_IDXS = TOK * HG                # 8192 indices per gather instruction
SLOTS = NUM_IDXS // 128            # 64 slots -> [128, 64, 12] gather tile
GTOK = TOK // 128                  # 8 token groups per instruction
N_BLOCKS = T // TOK                # 16 token blocks per core
S = NUM_IDXS // 16                 # wrapped idx free length (512)


def _dma_gather_raw(nc, out_ap, in_ap, idxs_ap, num_idxs, elem_size, elem_step):
    gp = nc.gpsimd
    stride_bytes_256 = exact_div(elem_step * mybir.dt.size(in_ap.dtype), 256)
    _in_ap = gp.lower_ap_dma(in_ap, for_custom_bir_dma=True)
    _idxs_ap = gp.lower_ap(idxs_ap)
    _out_ap = gp.lower_ap(out_ap)
    return gp.add_instruction(
        mybir.InstDMAGatherAnt(
            name=nc.get_next_instruction_name(),
            ins=[*_in_ap, _idxs_ap, gp.lower_val_access(gp.to_reg(num_idxs))],
            outs=[_out_ap],
            transpose=False,
            num_idxs=num_idxs,
            elem_size=elem_size,
            stride_bytes_256=stride_bytes_256,
            gen_mode=0,
            single_packet=False,
            queue_num=0,
            sbuf_tokens_per_rank=0,
            sbuf_free_dim_per_rank=0,
            sbuf_free_dim_pad_per_rank=0,
            sbuf_byte_offset=0,
        )
    )


def _build_program():
    nc = bacc.Bacc(
        "TRN2",
        target_bir_lowering=False,
        debug=False,
        enable_asserts=False,
        num_devices=N_CORES,
    )
    # wrapped idx: one [P, S] slab per (block, head-group) instruction
    idxw = nc.dram_tensor(
        "idxw", [N_BLOCKS * HG, P, S], mybir.dt.int16, kind="ExternalInput"
    )
    table = nc.dram_tensor(
        "table", [H * C, PAD], mybir.dt.float32, kind="ExternalInput"
    )
    out = nc.dram_tensor("out", [T, F], mybir.dt.float32, kind="ExternalOutput")

    with tile.TileContext(nc) as tc:
        nc.gpsimd.load_library(library_config.mlp)
        with (
            tc.tile_pool(name="idxp", bufs=4) as idx_pool,
            tc.tile_pool(name="datap", bufs=4) as data_pool,
        ):
            for blk in range(N_BLOCKS):
                tok_base = blk * TOK
                for hg in range(HG):
                    inst = blk * HG + hg
                    idx_tile = idx_pool.tile([P, S], mybir.dt.int16, tag="idx")
                    nc.sync.dma_start(out=idx_tile[:], in_=idxw[inst, :, :])
                    data_tile = data_pool.tile(
                        [P, SLOTS * D], mybir.dt.float32, tag="data"
                    )
                    _dma_gather_raw(
                        nc,
                        out_ap=data_tile[:].rearrange("p (s d) -> p s d", d=D),
                        in_ap=table[hg * HG * C : (hg + 1) * HG * C, 0:D],
                        idxs_ap=idx_tile[:],
                        num_idxs=NUM_IDXS,
                        elem_size=D,
                        elem_step=PAD,
                    )
                    # token (g, p) holds its 8-head segment (96 f32) at
                    # free offset g*96; store per token-group g.
                    for g in range(GTOK):
                        nc.sync.dma_start(
                            out=out[
                                tok_base + g * 128 : tok_base + (g + 1) * 128,
                                hg * HG * D : (hg + 1) * HG * D,
                            ],
                            in_=data_tile[:, g * HG * D : (g + 1) * HG * D],
                        )
    nc.compile()
    return nc


_NC_CACHE = None


def _get_program():
    global _NC_CACHE
    if _NC_CACHE is None:
        _NC_CACHE = _build_program()
    return _NC_CACHE


def _marshal_core_indices(idx_core):
    """idx_core: (T, H) int32 codebook ids. Returns wrapped int16
    [N_BLOCKS*HG, P, S] in the ucode's consumption order."""
    # j = (g*8 + hh)*128 + p within an instruction
    j = np.arange(NUM_IDXS)
    g = j // (HG * 128)
    hh = (j // 128) % HG
    p = j % 128
    out = np.empty((N_BLOCKS, HG, NUM_IDXS), dtype=np.int16)
    for blk in range(N_BLOCKS):
        token = blk * TOK + g * 128 + p  # (NUM_IDXS,)
        for hg in range(HG):
            head = hg * HG + hh
            out[blk, hg] = (idx_core[token, head] + hh * C).astype(np.int16)
    # wrap: index j -> [j % 16, j // 16]; replicate across 8 core groups
    wrapped = out.reshape(N_BLOCKS, HG, S, 16).transpose(0, 1, 3, 2)  # [.., 16, S]
    rep = np.broadcast_to(
        wrapped[:, :, None, :, :], (N_BLOCKS, HG, 8, 16, S)
    ).reshape(N_BLOCKS * HG, P, S)
    return np.ascontiguousarray(rep)


def kernel(memory_indices, values):
    memory_indices = np.asarray(memory_indices)
    values_np = np.asarray(values, dtype=np.float32)

    table = np.zeros((H * C, PAD), dtype=np.float32)
    table[:, :D] = values_np.reshape(H * C, D)

    idx32 = memory_indices.reshape(B, N, H).astype(np.int32)

    nc = _get_program()
    in_maps = []
    for c in range(N_CORES):
        idx_core = idx32[c * B_PER_CORE : (c + 1) * B_PER_CORE].reshape(T, H)
        in_maps.append(
            {"idxw": _marshal_core_indices(idx_core), "table": table}
        )
    res = run_bass_kernel_spmd(nc, in_maps, core_ids=list(range(N_CORES)))
    out = np.empty((B, N, F), dtype=np.float32)
    for c in range(N_CORES):
        out[c * B_PER_CORE : (c + 1) * B_PER_CORE] = res.results[c][
            "out"
        ].reshape(B_PER_CORE, N, F)
    return out
